# revision 16
# baseline (speedup 1.0000x reference)
"""Trainium2 Bass kernel for an autoregressive GRU decoder.

Reference semantics (per row of a [B*A, .] batch, T sequential steps):
    h0 = tanh(W_lat @ lat + b_lat)
    x0 = inputs[:, :, 0, :]          # later timesteps of `inputs` are unused
    per step:
        xe = W_emb @ x + b_emb
        gx = W_ih @ xe + b_ih ; gh = W_hh @ h + b_hh
        r = sig(gxr+ghr); z = sig(gxz+ghz); n = tanh(gxn + r*ghn)
        h' = (1-z)*n + z*h
        x' = x + W_out @ h' + b_out
    output = stack of x_t, shape [B, A, T, n_in]

Device strategy (8 NeuronCores, data-parallel over B*A = 2048 rows,
256 rows per core):
  - W_emb is folded into W_ih on the host: W_ihe = W_ih @ W_emb [1536, 64],
    so the embed matmul disappears and the input-gate matmul shrinks to K=64.
  - Everything on device is feature-major: rows live on the matmul moving
    dimension (N=256), features on partitions. No transposes anywhere.
  - Matmuls run in native fp32. The recurrence is chaotic (~700x error
    amplification over 127 steps), so reduced-precision matmul modes
    (tf32/bf16) drift 4e-2..2e-1 from the fp32 reference — fp32 is the
    only numerically safe choice.
  - r/z-gate biases ride a constant ones-row appended to the x tile
    (row 64); b_hh[n-gate] is fused into the DVE scalar_tensor_tensor that
    forms r*(ghn+b); b_out is fused into the x-update scalar_tensor_tensor.
  - PSUM: 4 tiles of [128,1024] (2 banks each) hold r / z / ghn / (gxn,xout)
    gate pre-activations; one accumulation group per 2KB bank (start on the
    first matmul touching the bank, stop on the last).
  - t=0 of the output equals x0 and is filled in on the host.
"""

import os
import sys

import numpy as np

if "/opt/trn_rl_repo" not in sys.path:
    sys.path.insert(0, "/opt/trn_rl_repo")

B, A, T = 32, 64, 128
NIN, NLAT, NEMB, NHID = 64, 64, 256, 512
NG = 3 * NHID  # 1536
NCORES = 8
R = (B * A) // NCORES  # 256 rows per core
KC = NHID // 128  # 4 hid chunks

PROFILE = False
LAST_RESULT = None  # BassKernelResults of the most recent run (for test.py)

_PROGRAM_CACHE = {}


def _build(t_steps):
    import concourse.bass as bass
    import concourse.mybir as mybir
    from concourse import tile

    F32 = mybir.dt.float32
    AF = mybir.ActivationFunctionType
    OP = mybir.AluOpType

    nc = bass.Bass()

    whh_d = nc.dram_tensor("whh", [128, KC * NG], F32, kind="ExternalInput")
    wihe_d = nc.dram_tensor("wihe", [NIN + 1, NG], F32, kind="ExternalInput")
    wout_d = nc.dram_tensor("wout", [128, KC * NIN], F32, kind="ExternalInput")
    wlat_d = nc.dram_tensor("wlat", [NLAT + 1, NHID], F32, kind="ExternalInput")
    bhhn_d = nc.dram_tensor("bhhn", [128, KC], F32, kind="ExternalInput")
    bout_d = nc.dram_tensor("bout", [NIN, 1], F32, kind="ExternalInput")
    latT_d = nc.dram_tensor("latT", [NLAT + 1, R], F32, kind="ExternalInput")
    x0T_d = nc.dram_tensor("x0T", [NIN + 1, R], F32, kind="ExternalInput")
    out_d = nc.dram_tensor("out", [t_steps, NIN, R], F32, kind="ExternalOutput")

    with tile.TileContext(nc) as tc:
        with (
            tc.tile_pool(name="const", bufs=1) as cpool,
            tc.tile_pool(name="state", bufs=1) as spool,
            tc.tile_pool(name="work", bufs=2) as wpool,
            tc.tile_pool(name="ps", bufs=1, space="PSUM") as ppool,
        ):
            whh = cpool.tile_from(whh_d[:], name="whh_s")
            wihe = cpool.tile_from(wihe_d[:], name="wihe_s")
            wout = cpool.tile_from(wout_d[:], name="wout_s")
            wlat = cpool.tile_from(wlat_d[:], name="wlat_s")
            bhhn = cpool.tile_from(bhhn_d[:], name="bhhn_s")
            bout = cpool.tile_from(bout_d[:], name="bout_s")

            # x state: rows 0..63 data (fp32 master, updated in place),
            # row 64 constant ones (bias fold for the W_ihe matmuls).
            x_t = spool.tile([NIN + 1, R], F32, name="x_t")
            h_t = spool.tile([128, KC * R], F32, name="h_t")

            nc.sync.dma_start(out=x_t[:], in_=x0T_d[:])

            def mm(out_ap, lhsT_ap, rhs_ap, start, stop):
                nc.tensor.matmul(out_ap, lhsT_ap, rhs_ap, start=start, stop=stop)

            # lhsT for W_hh gate j-tile jg (0..11), hid chunk k (0..3)
            def whh_l(jg, k):
                return whh[:, k * NG + jg * 128 : k * NG + (jg + 1) * 128]

            # ---- h0 = tanh(W_lat @ lat + b_lat) ----
            lat_t = wpool.tile([NLAT + 1, R], F32, tag="lat", name="lat_t")
            nc.sync.dma_start(out=lat_t[:], in_=latT_d[:])
            h0ps = ppool.tile([128, KC * R], F32, tag="rp", name="h0ps")
            for g in range(KC):
                mm(
                    h0ps[:, g * R : (g + 1) * R],
                    wlat[:, g * 128 : (g + 1) * 128],
                    lat_t[:],
                    start=(g % 2 == 0),
                    stop=(g % 2 == 1),
                )
            nc.scalar.activation(h_t[:], h0ps[:], AF.Tanh)

            for step in range(1, t_steps):
                # gxn = W_ihe[n] @ x (+ b_ihn via ones row): 4 single-matmul
                # groups in the xp tile (j-pairs share a PSUM bank).
                xp = ppool.tile([128, KC * R], F32, tag="xp", name="xp")
                for jj in range(4):
                    mm(
                        xp[:, jj * R : (jj + 1) * R],
                        wihe[:, (8 + jj) * 128 : (9 + jj) * 128],
                        x_t[:],
                        start=(jj % 2 == 0),
                        stop=(jj % 2 == 1),
                    )

                # ghn = W_hh[n] @ h  (b_hhn is fused into the t_t stt below)
                gp = ppool.tile([128, KC * R], F32, tag="gp", name="gp")
                for k in range(KC):
                    for jj in range(4):
                        mm(
                            gp[:, jj * R : (jj + 1) * R],
                            whh_l(8 + jj, k),
                            h_t[:, k * R : (k + 1) * R],
                            start=(k == 0 and jj % 2 == 0),
                            stop=(k == KC - 1 and jj % 2 == 1),
                        )

                # r / z pre-activations: sum of W_hh part and W_ihe part
                # (bias folded into the x ones-row).
                rp = ppool.tile([128, KC * R], F32, tag="rp", name="rp")
                zp = ppool.tile([128, KC * R], F32, tag="zp", name="zp")
                for k in range(KC):
                    for jj in range(4):
                        mm(
                            rp[:, jj * R : (jj + 1) * R],
                            whh_l(jj, k),
                            h_t[:, k * R : (k + 1) * R],
                            start=(k == 0 and jj % 2 == 0),
                            stop=False,
                        )
                    for jj in range(4):
                        mm(
                            zp[:, jj * R : (jj + 1) * R],
                            whh_l(4 + jj, k),
                            h_t[:, k * R : (k + 1) * R],
                            start=(k == 0 and jj % 2 == 0),
                            stop=False,
                        )
                for jj in range(4):
                    mm(
                        rp[:, jj * R : (jj + 1) * R],
                        wihe[:, jj * 128 : (jj + 1) * 128],
                        x_t[:],
                        start=False,
                        stop=(jj % 2 == 1),
                    )
                for jj in range(4):
                    mm(
                        zp[:, jj * R : (jj + 1) * R],
                        wihe[:, (4 + jj) * 128 : (5 + jj) * 128],
                        x_t[:],
                        start=False,
                        stop=(jj % 2 == 1),
                    )

                # gate math
                r_t = wpool.tile([128, KC * R], F32, tag="r", name="r_t")
                nc.scalar.activation(r_t[:], rp[:], AF.Sigmoid)
                z_t = wpool.tile([128, KC * R], F32, tag="z", name="z_t")
                nc.scalar.activation(z_t[:], zp[:], AF.Sigmoid)
                # t = r * (ghn + b_hhn), per hid-chunk so the bias can be a
                # per-partition scalar operand
                t_t = wpool.tile([128, KC * R], F32, tag="t", name="t_t")
                for c in range(KC):
                    sl = slice(c * R, (c + 1) * R)
                    nc.vector.scalar_tensor_tensor(
                        t_t[:, sl], gp[:, sl], bhhn[:, c : c + 1], r_t[:, sl],
                        OP.add, OP.mult,
                    )
                s_t = wpool.tile([128, KC * R], F32, tag="s", name="s_t")
                nc.vector.tensor_tensor(s_t[:], t_t[:], xp[:], OP.add)
                n_t = wpool.tile([128, KC * R], F32, tag="n", name="n_t")
                nc.scalar.activation(n_t[:], s_t[:], AF.Tanh)

                # h' = n + z*(h-n), in halves so the first two hid chunks
                # unblock next step's matmuls early.
                d_t = wpool.tile([128, KC * R], F32, tag="d", name="d_t")
                e_t = wpool.tile([128, KC * R], F32, tag="e", name="e_t")
                H = KC * R // 2
                for hf in range(2):
                    sl = slice(hf * H, (hf + 1) * H)
                    nc.vector.tensor_tensor(d_t[:, sl], h_t[:, sl], n_t[:, sl], OP.subtract)
                    nc.vector.tensor_tensor(e_t[:, sl], z_t[:, sl], d_t[:, sl], OP.mult)
                    nc.vector.tensor_tensor(h_t[:, sl], n_t[:, sl], e_t[:, sl], OP.add)

                # x' = x + W_out @ h' + b_out (in place); out[step] = x'
                xo = ppool.tile([NIN, R], F32, tag="xp", name="xo")
                for g in range(KC):
                    mm(
                        xo[:],
                        wout[:, g * NIN : (g + 1) * NIN],
                        h_t[:, g * R : (g + 1) * R],
                        start=(g == 0),
                        stop=(g == KC - 1),
                    )
                nc.vector.scalar_tensor_tensor(
                    x_t[0:NIN, :], xo[:], bout[:], x_t[0:NIN, :], OP.add, OP.add
                )
                nc.sync.dma_start(out=out_d[step], in_=x_t[0:NIN, :])

    return nc


def _fix_wait_overflow(nc):
    """Split semaphore waits that exceed per-instruction ISA capacity.

    walrus rejects fp32 matmuls (fused weight load) with >1 sync wait and
    DMAs with >2. Excess waits move to a same-engine InstDrain inserted
    immediately before the instruction — the engine is in-order, so the
    stall point is unchanged. (Tile's own kernel-tail drains carry 10+
    waits, so drains have no such capacity limit.)
    """
    import concourse.mybir as mybir

    caps = {"InstMatmult": 1, "InstDMACopy": 1, "InstTensorScalarPtr": 1,
            "InstTensorTensor": 1, "InstActivation": 1, "InstMemset": 1,
            "InstTensorCopy": 1, "InstTensorScalar": 1, "InstDrain": 1}
    for f in nc.m.functions:
        for blk in f.blocks:
            insts = list(blk.instructions)
            out = []
            changed = False
            for inst in insts:
                si = inst.sync_info
                ow = list(si.on_wait) if si and si.on_wait else []
                cap = caps.get(type(inst).__name__)
                if cap is not None and len(ow) > cap:
                    excess = ow[cap:]
                    dcap = caps["InstDrain"]
                    for i in range(0, len(excess), dcap):
                        d = mybir.InstDrain(
                            name=nc.get_next_instruction_name(),
                            ins=[],
                            outs=[],
                            bass_is_fusable=False,
                        )
                        d.engine = inst.engine
                        d.sync_info = mybir.SyncInfo(
                            on_wait=excess[i : i + dcap], on_update=[]
                        )
                        out.append(d)
                    inst.sync_info = mybir.SyncInfo(
                        on_wait=ow[:cap],
                        on_update=list(si.on_update) if si.on_update else [],
                    )
                    changed = True
                out.append(inst)
            if changed:
                blk.instructions = out
    return nc


def _get_program(t_steps):
    if t_steps not in _PROGRAM_CACHE:
        _PROGRAM_CACHE[t_steps] = _fix_wait_overflow(_build(t_steps))
    return _PROGRAM_CACHE[t_steps]


def _host_prep(latents, inputs, W_lat, b_lat, W_emb, b_emb, W_out, b_out, W_ih, b_ih, W_hh, b_hh):
    f32 = np.float32
    f64 = np.float64
    lat = np.asarray(latents, f32).reshape(B * A, NLAT)
    x0 = np.ascontiguousarray(np.asarray(inputs, f32)[:, :, 0, :]).reshape(B * A, NIN)

    W_ih64 = np.asarray(W_ih, f64)
    W_ihe = (W_ih64 @ np.asarray(W_emb, f64)).astype(f32)  # [1536, 64]
    b_row = (W_ih64 @ np.asarray(b_emb, f64) + np.asarray(b_ih, f64)).astype(f32)
    b_row[: 2 * NHID] += np.asarray(b_hh, f32)[: 2 * NHID]  # r,z gates get b_hh too

    whh = np.ascontiguousarray(
        np.asarray(W_hh, f32).T.reshape(KC, 128, NG).transpose(1, 0, 2).reshape(128, KC * NG)
    )
    wihe = np.empty((NIN + 1, NG), f32)
    wihe[:NIN] = W_ihe.T
    wihe[NIN] = b_row
    wout = np.ascontiguousarray(
        np.asarray(W_out, f32).T.reshape(KC, 128, NIN).transpose(1, 0, 2).reshape(128, KC * NIN)
    )
    wlat = np.empty((NLAT + 1, NHID), f32)
    wlat[:NLAT] = np.asarray(W_lat, f32).T
    wlat[NLAT] = np.asarray(b_lat, f32)
    # b_hh[n-gate] as per-partition scalars, one column per hid chunk
    bhhn = np.ascontiguousarray(
        np.asarray(b_hh, f32)[2 * NHID :].reshape(KC, 128).T
    )
    bout = np.ascontiguousarray(np.asarray(b_out, f32)[:, None])

    shared = dict(whh=whh, wihe=wihe, wout=wout, wlat=wlat, bhhn=bhhn, bout=bout)
    in_maps = []
    for c in range(NCORES):
        sl = slice(c * R, (c + 1) * R)
        latT = np.empty((NLAT + 1, R), f32)
        latT[:NLAT] = lat[sl].T
        latT[NLAT] = 1.0
        x0T = np.empty((NIN + 1, R), f32)
        x0T[:NIN] = x0[sl].T
        x0T[NIN] = 1.0
        in_maps.append(dict(shared, latT=latT, x0T=x0T))
    return in_maps


def kernel(**inputs):
    global LAST_RESULT
    from concourse import bass_utils

    in_maps = _host_prep(**inputs)
    nc = _get_program(T)
    kwargs = {}
    if PROFILE:
        kwargs = dict(trace=True, trace_cores=[0])
    res = bass_utils.run_bass_kernel_spmd(nc, in_maps, list(range(NCORES)), **kwargs)
    LAST_RESULT = res

    # per-core out is [T, NIN, R] -> rows-major [R, T, NIN]
    parts = [res.results[c]["out"].transpose(2, 0, 1) for c in range(NCORES)]
    full = np.concatenate(parts, axis=0)  # [B*A, T, NIN]
    out = full.reshape(B, A, T, NIN).astype(np.float32, copy=True)
    # t=0 of the output is exactly x0; the device never writes that slot.
    out[:, :, 0, :] = np.asarray(inputs["inputs"], np.float32)[:, :, 0, :]
    return out
